# revision 3
# baseline (speedup 1.0000x reference)
"""GraphVAE forward on 8 Trainium2 NeuronCores (Bass/Tile SPMD kernel).

Strategy:
- Nodes sharded 8-way by contiguous ranges (6250/core, padded to 6272 = 49
  blocks of 128). Edges assigned to the core owning their *dst* node, sorted
  by dst, grouped per 128-node block, padded to CH chunks of 128 edges.
- GCN aggregation per block: indirect-DMA gather of source-node rows,
  equality-matrix (dst_rel == iota) built on VectorE, segment-sum via PE
  matmul accumulation into PSUM. Symmetric norm dis[src]*dis[dst] is
  separable: source side pre-folded into the gathered table, dst side
  applied per-partition after aggregation.
- conv1 aggregates raw 32-dim x (gather is descriptor-bound; fewer bytes),
  then multiplies by W1. conv2 gathers 128-dim h1 rows from an AllGathered
  shared table.
- Graph mean via mask-vector matmul accumulated across blocks + AllReduce.
- Decoder: Wd2 host-rearranged to [128, NB, 32, 128] blocks so each
  [128,128] stationary operand is contiguous; 32 matmuls per node block.
"""
import sys

sys.path.insert(0, "/opt/trn_rl_repo")

import numpy as np
import concourse.tile as tile
from concourse import bass, bacc, mybir
from concourse import bass_utils

N = 50000
E = 1600000
C = 32
HID = 128
LAT = 64
NCORES = 8
SH = N // NCORES          # 6250 real nodes per core
NB = (SH + 127) // 128    # 49 blocks
SHP = NB * 128            # 6272 padded nodes per core
V = NCORES * SHP          # 50176 rows in padded global tables

f32 = mybir.dt.float32
i32 = mybir.dt.int32


def _build(CH: int):
    nc = bacc.Bacc("TRN2", target_bir_lowering=False, debug=False,
                   num_devices=NCORES)
    dt = nc.dram_tensor
    t_xs = dt("xs", [V, C], f32, kind="ExternalInput")
    t_srci = dt("srci", [NB, 128, CH], i32, kind="ExternalInput")
    t_dstr = dt("dstr", [NB, 128, CH], f32, kind="ExternalInput")
    t_dis = dt("dis", [NB, 128], f32, kind="ExternalInput")
    t_mask = dt("mask", [NB, 128], f32, kind="ExternalInput")
    t_w1 = dt("w1", [C, HID], f32, kind="ExternalInput")
    t_w2 = dt("w2", [HID, HID], f32, kind="ExternalInput")
    t_b1t = dt("b1t", [128, HID], f32, kind="ExternalInput")
    t_b2t = dt("b2t", [128, HID], f32, kind="ExternalInput")
    t_wmu = dt("wmu", [HID, LAT], f32, kind="ExternalInput")
    t_wlv = dt("wlv", [HID, LAT], f32, kind="ExternalInput")
    t_bmu = dt("bmu", [LAT, 1], f32, kind="ExternalInput")
    t_blv = dt("blv", [LAT, 1], f32, kind="ExternalInput")
    t_wd1 = dt("wd1", [LAT, HID], f32, kind="ExternalInput")
    t_bd1 = dt("bd1", [HID, 1], f32, kind="ExternalInput")
    t_eps = dt("eps", [LAT, 1], f32, kind="ExternalInput")
    t_wd2 = dt("wd2", [HID, NB * C * 128], f32, kind="ExternalInput")
    t_bd2 = dt("bd2", [NB, 128, C], f32, kind="ExternalInput")
    t_iota = dt("iota", [128, 128], f32, kind="ExternalInput")
    t_ident = dt("ident", [128, 128], f32, kind="ExternalInput")

    o_recon = dt("recon", [SH, C], f32, kind="ExternalOutput")
    o_mu = dt("mu", [LAT, 1], f32, kind="ExternalOutput")
    o_lv = dt("lv", [LAT, 1], f32, kind="ExternalOutput")

    d_h1s = dt("h1s", [SHP, HID], f32)
    d_h1f = dt("h1f", [V, HID], f32, addr_space="Shared")
    d_gin = dt("gin", [1, HID], f32)
    d_gout = dt("gout", [1, HID], f32, addr_space="Shared")

    rg = [list(range(NCORES))]
    AF = mybir.ActivationFunctionType
    OP = mybir.AluOpType

    with tile.TileContext(nc) as tc:
        cp = tc.alloc_tile_pool(name="const", bufs=1)
        sb = tc.alloc_tile_pool(name="sb", bufs=3)
        sS = tc.alloc_tile_pool(name="sS", bufs=4)
        ps = tc.alloc_tile_pool(name="ps", bufs=6, space="PSUM")
        pg = tc.alloc_tile_pool(name="pg", bufs=1, space="PSUM")

        iota = cp.tile([128, 128], f32)
        nc.sync.dma_start(out=iota[:], in_=t_iota[:, :])
        ident = cp.tile([128, 128], f32)
        nc.sync.dma_start(out=ident[:], in_=t_ident[:, :])
        w1 = cp.tile([C, HID], f32)
        nc.sync.dma_start(out=w1[:], in_=t_w1[:, :])
        w2 = cp.tile([HID, HID], f32)
        nc.sync.dma_start(out=w2[:], in_=t_w2[:, :])
        b1t = cp.tile([128, HID], f32)
        nc.sync.dma_start(out=b1t[:], in_=t_b1t[:, :])
        b2t = cp.tile([128, HID], f32)
        nc.sync.dma_start(out=b2t[:], in_=t_b2t[:, :])

        # ---- conv1: aggregate xs (pre-scaled by dis[src]) over edges ----
        for b in range(NB):
            idx_t = sb.tile([128, CH], i32, tag="idx")
            nc.sync.dma_start(out=idx_t[:], in_=t_srci[b, :, :])
            dstr_t = sb.tile([128, CH], f32, tag="dstr")
            nc.sync.dma_start(out=dstr_t[:], in_=t_dstr[b, :, :])
            dis_t = sb.tile([128, 1], f32, tag="dis")
            nc.sync.dma_start(out=dis_t[:], in_=t_dis[b, :, None])
            gath = sb.tile([128, CH * C], f32, tag="gath1")
            nc.gpsimd.indirect_dma_start(
                out=gath[:], out_offset=None, in_=t_xs[:, :],
                in_offset=bass.IndirectOffsetOnAxis(ap=idx_t[:, :], axis=0))
            ps1 = ps.tile([128, C], f32, tag="ps", space="PSUM")
            for j in range(CH):
                S = sS.tile([128, 128], f32, tag="S")
                nc.vector.tensor_tensor(
                    out=S[:], in0=dstr_t[:, j:j + 1].to_broadcast([128, 128]),
                    in1=iota[:], op=OP.is_equal)
                nc.tensor.matmul(out=ps1[:], lhsT=S[:],
                                 rhs=gath[:, j * C:(j + 1) * C],
                                 start=(j == 0), stop=(j == CH - 1))
            t32 = sb.tile([128, C], f32, tag="t32")
            nc.vector.tensor_scalar_mul(out=t32[:], in0=ps1[:],
                                        scalar1=dis_t[:, :1])
            t32T_ps = ps.tile([C, 128], f32, tag="ps", space="PSUM")
            nc.tensor.transpose(out=t32T_ps[:], in_=t32[:], identity=ident[:])
            t32T = sb.tile([C, 128], f32, tag="t32T")
            nc.vector.tensor_copy(out=t32T[:], in_=t32T_ps[:])
            psh1 = ps.tile([128, HID], f32, tag="ps", space="PSUM")
            nc.tensor.matmul(out=psh1[:], lhsT=t32T[:], rhs=w1[:],
                             start=True, stop=True)
            tmp1 = sb.tile([128, HID], f32, tag="tmp1")
            nc.vector.tensor_tensor(out=tmp1[:], in0=psh1[:], in1=b1t[:],
                                    op=OP.add)
            h1s_t = sb.tile([128, HID], f32, tag="h1s")
            nc.vector.tensor_scalar(out=h1s_t[:], in0=tmp1[:], scalar1=0.0,
                                    scalar2=dis_t[:, :1], op0=OP.max,
                                    op1=OP.mult)
            nc.sync.dma_start(out=d_h1s[b * 128:(b + 1) * 128, :],
                              in_=h1s_t[:])

        # ---- allgather h1s shards into the shared table ----
        nc.gpsimd.collective_compute(
            "AllGather", OP.bypass, ins=[d_h1s[:, :]], outs=[d_h1f[:, :]],
            replica_groups=rg)

        # ---- conv2 + masked sum for graph mean ----
        gacc = pg.tile([1, HID], f32, tag="gacc", space="PSUM")
        for b in range(NB):
            idx_t = sb.tile([128, CH], i32, tag="idx")
            nc.sync.dma_start(out=idx_t[:], in_=t_srci[b, :, :])
            dstr_t = sb.tile([128, CH], f32, tag="dstr")
            nc.sync.dma_start(out=dstr_t[:], in_=t_dstr[b, :, :])
            dis_t = sb.tile([128, 1], f32, tag="dis")
            nc.sync.dma_start(out=dis_t[:], in_=t_dis[b, :, None])
            mask_t = sb.tile([128, 1], f32, tag="mask")
            nc.sync.dma_start(out=mask_t[:], in_=t_mask[b, :, None])
            gath2 = sb.tile([128, CH * HID], f32, tag="gath2")
            nc.gpsimd.indirect_dma_start(
                out=gath2[:], out_offset=None, in_=d_h1f[:, :],
                in_offset=bass.IndirectOffsetOnAxis(ap=idx_t[:, :], axis=0))
            ps2 = ps.tile([128, HID], f32, tag="ps", space="PSUM")
            for j in range(CH):
                S = sS.tile([128, 128], f32, tag="S")
                nc.vector.tensor_tensor(
                    out=S[:], in0=dstr_t[:, j:j + 1].to_broadcast([128, 128]),
                    in1=iota[:], op=OP.is_equal)
                nc.tensor.matmul(out=ps2[:], lhsT=S[:],
                                 rhs=gath2[:, j * HID:(j + 1) * HID],
                                 start=(j == 0), stop=(j == CH - 1))
            t2 = sb.tile([128, HID], f32, tag="t2")
            nc.vector.tensor_scalar_mul(out=t2[:], in0=ps2[:],
                                        scalar1=dis_t[:, :1])
            t2T_ps = ps.tile([128, 128], f32, tag="ps", space="PSUM")
            nc.tensor.transpose(out=t2T_ps[:], in_=t2[:], identity=ident[:])
            t2T = sb.tile([128, 128], f32, tag="t2T")
            nc.vector.tensor_copy(out=t2T[:], in_=t2T_ps[:])
            psh2 = ps.tile([128, HID], f32, tag="ps", space="PSUM")
            nc.tensor.matmul(out=psh2[:], lhsT=t2T[:], rhs=w2[:],
                             start=True, stop=True)
            tmp2 = sb.tile([128, HID], f32, tag="tmp2")
            nc.vector.tensor_tensor(out=tmp2[:], in0=psh2[:], in1=b2t[:],
                                    op=OP.add)
            h2 = sb.tile([128, HID], f32, tag="h2")
            nc.scalar.activation(out=h2[:], in_=tmp2[:], func=AF.Relu)
            nc.tensor.matmul(out=gacc[:], lhsT=mask_t[:, :1], rhs=h2[:],
                             start=(b == 0), stop=(b == NB - 1))

        # ---- graph mean, mu/logvar, z, d ----
        g_row = sb.tile([1, HID], f32, tag="g_row")
        nc.vector.tensor_copy(out=g_row[:], in_=gacc[:])
        nc.sync.dma_start(out=d_gin[:, :], in_=g_row[:])
        nc.gpsimd.collective_compute(
            "AllReduce", OP.add, ins=[d_gin[:, :]], outs=[d_gout[:, :]],
            replica_groups=rg)
        g_col = sb.tile([HID, 1], f32, tag="g_col")
        nc.sync.dma_start(out=g_col[:], in_=d_gout[0, :, None])
        gs = sb.tile([HID, 1], f32, tag="gs")
        nc.vector.tensor_scalar_mul(out=gs[:], in0=g_col[:], scalar1=1.0 / N)

        wmu = cp.tile([HID, LAT], f32)
        nc.sync.dma_start(out=wmu[:], in_=t_wmu[:, :])
        wlv = cp.tile([HID, LAT], f32)
        nc.sync.dma_start(out=wlv[:], in_=t_wlv[:, :])
        bmu = cp.tile([LAT, 1], f32)
        nc.sync.dma_start(out=bmu[:], in_=t_bmu[:, :])
        blv = cp.tile([LAT, 1], f32)
        nc.sync.dma_start(out=blv[:], in_=t_blv[:, :])
        wd1 = cp.tile([LAT, HID], f32)
        nc.sync.dma_start(out=wd1[:], in_=t_wd1[:, :])
        bd1 = cp.tile([HID, 1], f32)
        nc.sync.dma_start(out=bd1[:], in_=t_bd1[:, :])
        epsc = cp.tile([LAT, 1], f32)
        nc.sync.dma_start(out=epsc[:], in_=t_eps[:, :])

        mu_ps = ps.tile([LAT, 1], f32, tag="ps", space="PSUM")
        nc.tensor.matmul(out=mu_ps[:], lhsT=wmu[:], rhs=gs[:], start=True,
                         stop=True)
        mu_sb = sb.tile([LAT, 1], f32, tag="mu")
        nc.vector.tensor_tensor(out=mu_sb[:], in0=mu_ps[:], in1=bmu[:],
                                op=OP.add)
        lv_ps = ps.tile([LAT, 1], f32, tag="ps", space="PSUM")
        nc.tensor.matmul(out=lv_ps[:], lhsT=wlv[:], rhs=gs[:], start=True,
                         stop=True)
        lv_sb = sb.tile([LAT, 1], f32, tag="lv")
        nc.vector.tensor_tensor(out=lv_sb[:], in0=lv_ps[:], in1=blv[:],
                                op=OP.add)
        nc.sync.dma_start(out=o_mu[:, :], in_=mu_sb[:])
        nc.sync.dma_start(out=o_lv[:, :], in_=lv_sb[:])

        e_sb = sb.tile([LAT, 1], f32, tag="e")
        nc.scalar.activation(out=e_sb[:], in_=lv_sb[:], func=AF.Exp,
                             scale=0.5)
        ze = sb.tile([LAT, 1], f32, tag="ze")
        nc.vector.tensor_tensor(out=ze[:], in0=e_sb[:], in1=epsc[:],
                                op=OP.mult)
        z = sb.tile([LAT, 1], f32, tag="z")
        nc.vector.tensor_tensor(out=z[:], in0=ze[:], in1=mu_sb[:], op=OP.add)
        d_ps = ps.tile([HID, 1], f32, tag="ps", space="PSUM")
        nc.tensor.matmul(out=d_ps[:], lhsT=wd1[:], rhs=z[:], start=True,
                         stop=True)
        dT = sb.tile([HID, 1], f32, tag="dT")
        nc.scalar.activation(out=dT[:], in_=d_ps[:], func=AF.Relu,
                             bias=bd1[:, :1])

        # ---- decoder: recon block = Wd2_blk.T @ d + bd2 ----
        for b in range(NB):
            wblk = sb.tile([128, C * 128], f32, tag="wblk")
            nc.sync.dma_start(out=wblk[:],
                              in_=t_wd2[:, b * C * 128:(b + 1) * C * 128])
            psd = ps.tile([128, C], f32, tag="ps", space="PSUM")
            for c in range(C):
                nc.tensor.matmul(out=psd[:, c:c + 1],
                                 lhsT=wblk[:, c * 128:(c + 1) * 128],
                                 rhs=dT[:], start=True, stop=True)
            bd2_t = sb.tile([128, C], f32, tag="bd2")
            nc.sync.dma_start(out=bd2_t[:], in_=t_bd2[b, :, :])
            rec = sb.tile([128, C], f32, tag="rec")
            nc.vector.tensor_tensor(out=rec[:], in0=psd[:], in1=bd2_t[:],
                                    op=OP.add)
            rows = min(128, SH - b * 128)
            nc.sync.dma_start(out=o_recon[b * 128:b * 128 + rows, :],
                              in_=rec[:rows, :])

        for p in (pg, ps, sS, sb, cp):
            p.release()

    nc.compile()
    return nc


def _prep(inputs):
    x = np.ascontiguousarray(np.asarray(inputs["x"], np.float32))
    ei = np.asarray(inputs["edge_index"]).astype(np.int64)
    eps = np.asarray(inputs["eps"], np.float32)
    W1 = np.asarray(inputs["W1"], np.float32)
    b1 = np.asarray(inputs["b1"], np.float32)
    W2 = np.asarray(inputs["W2"], np.float32)
    b2 = np.asarray(inputs["b2"], np.float32)
    Wmu = np.asarray(inputs["Wmu"], np.float32)
    bmu = np.asarray(inputs["bmu"], np.float32)
    Wlv = np.asarray(inputs["Wlv"], np.float32)
    blv = np.asarray(inputs["blv"], np.float32)
    Wd1 = np.asarray(inputs["Wd1"], np.float32)
    bd1 = np.asarray(inputs["bd1"], np.float32)
    Wd2 = np.asarray(inputs["Wd2"], np.float32)
    bd2 = np.asarray(inputs["bd2"], np.float32)

    src = np.concatenate([ei[0], np.arange(N, dtype=np.int64)])
    dst = np.concatenate([ei[1], np.arange(N, dtype=np.int64)])
    deg = np.bincount(dst, minlength=N).astype(np.float32)
    dis = 1.0 / np.sqrt(deg)

    order = np.argsort(dst, kind="stable")
    srcs = src[order]
    dsts = dst[order]

    core_of = dsts // SH
    loc = dsts - core_of * SH
    blk = loc // 128
    rel = (loc - blk * 128).astype(np.float32)
    gsrc = ((srcs // SH) * SHP + (srcs % SH)).astype(np.int32)

    key = (core_of * NB + blk).astype(np.int64)
    cnt = np.bincount(key, minlength=NCORES * NB)
    CH = int(np.ceil(cnt.max() / 128))
    starts = np.zeros(NCORES * NB, np.int64)
    starts[1:] = np.cumsum(cnt)[:-1]
    k_in = np.arange(len(dsts), dtype=np.int64) - starts[key]
    flat = key * (128 * CH) + (k_in % 128) * CH + (k_in // 128)

    src_idx = np.zeros(NCORES * NB * 128 * CH, np.int32)
    dst_rel = np.full(NCORES * NB * 128 * CH, -1.0, np.float32)
    src_idx[flat] = gsrc
    dst_rel[flat] = rel
    src_idx = src_idx.reshape(NCORES, NB, 128, CH)
    dst_rel = dst_rel.reshape(NCORES, NB, 128, CH)

    dis_dev = np.zeros((NCORES, SHP), np.float32)
    mask_dev = np.zeros((NCORES, SHP), np.float32)
    dis_dev[:, :SH] = dis.reshape(NCORES, SH)
    mask_dev[:, :SH] = 1.0
    dis_dev = dis_dev.reshape(NCORES, NB, 128)
    mask_dev = mask_dev.reshape(NCORES, NB, 128)

    xs_dev = np.zeros((V, C), np.float32)
    xs_dev.reshape(NCORES, SHP, C)[:, :SH, :] = \
        (x * dis[:, None]).reshape(NCORES, SH, C)

    iota_np = np.broadcast_to(
        np.arange(128, dtype=np.float32)[None, :], (128, 128)).copy()
    ident_np = np.eye(128, dtype=np.float32)
    b1t = np.broadcast_to(b1[None, :], (128, HID)).copy()
    b2t = np.broadcast_to(b2[None, :], (128, HID)).copy()

    wd2_all = Wd2.reshape(HID, N, C)
    bd2_all = bd2.reshape(N, C)

    in_maps = []
    common = {
        "xs": xs_dev, "w1": W1, "w2": W2, "b1t": b1t, "b2t": b2t,
        "wmu": Wmu, "wlv": Wlv, "bmu": bmu.reshape(LAT, 1),
        "blv": blv.reshape(LAT, 1), "wd1": Wd1, "bd1": bd1.reshape(HID, 1),
        "eps": eps.reshape(LAT, 1), "iota": iota_np, "ident": ident_np,
    }
    for ci in range(NCORES):
        wblk = np.zeros((HID, SHP, C), np.float32)
        wblk[:, :SH, :] = wd2_all[:, ci * SH:(ci + 1) * SH, :]
        wd2_dev = np.ascontiguousarray(
            wblk.reshape(HID, NB, 128, C).transpose(0, 1, 3, 2)
        ).reshape(HID, NB * C * 128)
        bd2_dev = np.zeros((SHP, C), np.float32)
        bd2_dev[:SH] = bd2_all[ci * SH:(ci + 1) * SH]
        m = dict(common)
        m.update({
            "srci": src_idx[ci], "dstr": dst_rel[ci], "dis": dis_dev[ci],
            "mask": mask_dev[ci], "wd2": wd2_dev,
            "bd2": bd2_dev.reshape(NB, 128, C),
        })
        in_maps.append(m)
    return CH, in_maps


_compiled = {}


def kernel(**inputs):
    import os
    CH, in_maps = _prep(inputs)
    if CH not in _compiled:
        _compiled[CH] = _build(CH)
    nc = _compiled[CH]
    trace = bool(os.environ.get("KERNEL_TRACE"))
    if trace:
        import ntff_shim  # noqa: F401
    res = bass_utils.run_bass_kernel_spmd(
        nc, in_maps, core_ids=list(range(NCORES)), trace=trace)
    if trace:
        print(f"HW exec time: {res.exec_time_ns} ns")
    recon = np.concatenate([r["recon"] for r in res.results], axis=0)
    mu = res.results[0]["mu"].reshape(LAT)
    lv = res.results[0]["lv"].reshape(LAT)
    return recon, mu, lv
